# revision 71
# baseline (speedup 1.0000x reference)
import sys
sys.path.insert(0, '/opt/trn_rl_repo')
import numpy as np
import ml_dtypes
import concourse.bass as bass
import concourse.tile as tile
from concourse import bacc, mybir
from concourse.bass_utils import run_bass_kernel_spmd
from concourse.masks import make_identity
from concourse import bass_isa

F32 = mybir.dt.float32
BF = mybir.dt.bfloat16
ALU = mybir.AluOpType
AFT = mybir.ActivationFunctionType
H, D, DIM, QG, N = 8, 64, 512, 8, 1024
EPS_LN, EPS_VAR = 1e-5, 1e-4
N_CORES = 8
NT = N // 128      # 8 token tiles
KT = DIM // 128    # 4 feature tiles
M_TOT = float(H * QG * N)
F32R = mybir.dt.float32r


def r32(ap):
    """View an fp32 AP as float32r: same bits, 4x matmul rate at >=256 free."""
    return ap.bitcast(F32R)




def build_bass(single_core=False, debug=False):
    ncores = 1 if single_core else N_CORES
    nc = bacc.Bacc("TRN2", target_bir_lowering=False, debug=False, num_devices=ncores)

    xq = nc.dram_tensor("xq", [N, DIM], F32, kind="ExternalInput").ap()
    xk = nc.dram_tensor("xk", [N, DIM], F32, kind="ExternalInput").ap()
    xv = nc.dram_tensor("xv", [N, DIM], F32, kind="ExternalInput").ap()
    wp_d = nc.dram_tensor("wp", [128, KT * DIM], BF, kind="ExternalInput").ap()
    cbc_d = nc.dram_tensor("cb_col", [DIM, 1], F32, kind="ExternalInput").ap()
    cbr_d = nc.dram_tensor("cb_row", [1, DIM], F32, kind="ExternalInput").ap()
    wout_d = nc.dram_tensor("wout", [128, KT * DIM], BF, kind="ExternalInput").ap()
    bout_d = nc.dram_tensor("bout_row", [1, DIM], F32, kind="ExternalInput").ap()
    gpk_d = nc.dram_tensor("gate_pack", [128, 72], F32, kind="ExternalInput").ap()
    sel_d = nc.dram_tensor("sel", [128, 2], F32R, kind="ExternalInput").ap()
    ones_d = nc.dram_tensor("ones_col", [128, 1], F32R, kind="ExternalInput").ap()
    e2_d = nc.dram_tensor("e2blk", [2, 128], BF, kind="ExternalInput").ap()
    out_d = nc.dram_tensor("out", [N, DIM], F32, kind="ExternalOutput").ap()
    if debug:
        dbg_fq = nc.dram_tensor("dbg_fq", [128, N], F32, kind="ExternalOutput").ap()
        dbg_fk = nc.dram_tensor("dbg_fk", [128, DIM], F32, kind="ExternalOutput").ap()
        dbg_fv = nc.dram_tensor("dbg_fv", [128, 2 * DIM], F32, kind="ExternalOutput").ap()
        dbg_nq = nc.dram_tensor("dbg_nq", [2, N], F32, kind="ExternalOutput").ap()
        dbg_mq = nc.dram_tensor("dbg_mq", [2, N], F32, kind="ExternalOutput").ap()
        dbg_gtb = nc.dram_tensor("dbg_gtb", [128, N], F32, kind="ExternalOutput").ap()


    with tile.TileContext(nc) as tc:
        from contextlib import ExitStack
        with ExitStack() as es:
            consts = es.enter_context(tc.tile_pool(name="consts", bufs=1))
            persist = es.enter_context(tc.tile_pool(name="persist", bufs=1))
            dram = es.enter_context(tc.tile_pool(name="dram", bufs=1, space="DRAM"))

            ident = consts.tile([128, 128], BF)
            make_identity(nc, ident)
            sel = consts.tile([128, 2], F32R)
            sel_bf = consts.tile([128, 2], BF)
            ones_col = consts.tile([128, 1], F32R)
            ones_colbf = consts.tile([128, 1], BF)
            nc.vector.memset(ones_colbf, 1.0)
            e2blk = consts.tile([2, 128], BF)
            eps_ln_t = consts.tile([128, 1], F32)
            nc.vector.memset(eps_ln_t, EPS_LN)
            eps_var_t = consts.tile([128, 1], F32)
            nc.vector.memset(eps_var_t, EPS_VAR)

            wp_all = consts.tile([128, KT * DIM], BF)
            nc.sync.dma_start(wp_all, wp_d)
            wp = [wp_all[:, t * DIM:(t + 1) * DIM] for t in range(KT)]
            bout_row = consts.tile([1, DIM], F32)
            ones_row_bf = consts.tile([1, 128], BF)
            nc.vector.memset(ones_row_bf, 1.0)
            bout_bf = consts.tile([1, DIM], BF)
            gpk = consts.tile([128, 72], F32)
            pw1 = gpk[:, 0:64]
            pb1 = gpk[0:64, 64:65]
            plng = gpk[0:64, 65:66]
            plnb = gpk[0:64, 66:67]
            pw2 = gpk[0:64, 67:68]
            pb2 = gpk[0:1, 68:69]

            # persistent activations
            fqT = [persist.tile([128, N], F32R, tag=f"fqT{t}", name=f"fqT{t}") for t in range(KT)]
            fk = [persist.tile([128, DIM], BF, tag=f"fk{t}", name=f"fk{t}") for t in range(NT)]
            fvall = [persist.tile([128, 2 * DIM], BF, tag=f"fvall{t}", name=f"fvall{t}") for t in range(NT)]
            GTb = [persist.tile([128, N], BF, tag=f"GTb{t}", name=f"GTb{t}") for t in range(KT)]
            mq_all = persist.tile([2, KT * N], F32R)
            nq_all = persist.tile([2, KT * N], BF)
            mqst = [mq_all[:, t * N:(t + 1) * N] for t in range(KT)]
            nqst = [nq_all[:, t * N:(t + 1) * N] for t in range(KT)]
            # per-token k stats (partition-aligned per token tile)
            invnk = [persist.tile([128, H], F32, tag=f"invnk{t}", name=f"invnk{t}") for t in range(NT)]
            pack128 = persist.tile([128, 16], F32)
            omx = persist.tile([128, KT], F32)
            w64x = persist.tile([128, KT], F32)
            zeros64 = persist.tile([64, 64], BF)
            nc.vector.memset(zeros64, 0.0)
            mlp_raw = persist.tile([128, H], F32)
            e2_raw = persist.tile([128, H], F32)
            w64_bc = persist.tile([128, H], F32)
            om_bc = persist.tile([128, H], F32)

            red_in = dram.tile([64, 32], F32)
            red_out = dram.tile([64, 32], F32)

            # ================= PHASE A =================
            with tc.tile_pool(name="xp", bufs=5) as xp, \
                 tc.tile_pool(name="yp", bufs=4) as yp, \
                 tc.tile_pool(name="yT", bufs=2) as yTp, \
                 tc.tile_pool(name="sc", bufs=7) as scp, \
                 tc.tile_pool(name="sqp", bufs=2) as sqp, \
                 tc.tile_pool(name="ps_tp", bufs=2, space="PSUM") as ps_tp, \
                 tc.tile_pool(name="ps_big", bufs=2, space="PSUM") as ps_big, \
                 tc.tile_pool(name="ps_stat", bufs=2, space="PSUM") as ps_stat:

                def preload(x_d, tag):
                    ts = []
                    for i in range(NT):
                        x_t = xp.tile([128, DIM], F32, tag=f"x{tag}{i}")
                        nc.sync.dma_start(x_t, x_d[i * 128:(i + 1) * 128, :])
                        ts.append(x_t)
                    return ts

                def stream(x_d):
                    ts = []
                    for i in range(NT):
                        x_t = xp.tile([128, DIM], F32, tag="xs")
                        nc.sync.dma_start(x_t, x_d[i * 128:(i + 1) * 128, :])
                        ts.append(x_t)
                    return ts

                def load_norm_transpose(xts, all_act=False, y_act=False):
                    """Interleaved transposed-LN: yTt[:, i*512 + c*128 : +128] =
                    (y tile i, feature chunk c)^T."""
                    yTt = yTp.tile([128, KT * N], BF, tag="yT")
                    for i in range(NT):
                        x_t = xts[i]
                        st6 = scp.tile([128, 6], F32, tag="st6")
                        nc.vector.bn_stats(st6, x_t)
                        mv = scp.tile([128, 2], F32, tag="mv")
                        nc.vector.bn_aggr(mv, st6)
                        sd = scp.tile([128, 1], F32, tag="sd")
                        nc.scalar.activation(sd, mv[:, 1:2], AFT.Sqrt, bias=eps_ln_t)
                        r = scp.tile([128, 1], F32, tag="r")
                        nc.vector.reciprocal(r, sd)
                        y_t = yp.tile([128, DIM], BF, tag="y")
                        if y_act:
                            negmur = scp.tile([128, 1], F32, tag="nmr")
                            nc.vector.tensor_scalar(out=negmur, in0=mv[:, 0:1],
                                                    scalar1=r, scalar2=-1.0,
                                                    op0=ALU.mult, op1=ALU.mult)
                            nc.scalar.activation(y_t, x_t, AFT.Identity,
                                                 bias=negmur, scale=r)
                        else:
                            nc.vector.tensor_scalar(out=y_t, in0=x_t, scalar1=mv[:, 0:1],
                                                    scalar2=r, op0=ALU.subtract, op1=ALU.mult)
                        pst = ps_tp.tile([128, DIM], BF, tag="tp")
                        for c in range(KT):
                            nc.tensor.transpose(pst[:, c * 128:(c + 1) * 128],
                                                y_t[:, c * 128:(c + 1) * 128], ident)
                        if i % 2 == 0 and not all_act:
                            nc.vector.tensor_copy(
                                out=yTt[:, i * DIM:(i + 1) * DIM], in_=pst)
                        else:
                            nc.scalar.copy(out=yTt[:, i * DIM:(i + 1) * DIM], in_=pst)
                    return yTt

                def project_N(yTt, fN, half=None):
                    """fN[nt][...] = y @ Wp  (partition = token)"""
                    for nt in range(NT):
                        ps = ps_big.tile([128, N], F32, tag="big")
                        for kt in range(KT):
                            nc.tensor.matmul(
                                ps[:, 0:DIM],
                                yTt[:, nt * DIM + kt * 128:nt * DIM + (kt + 1) * 128],
                                (wp[kt]), start=(kt == 0), stop=(kt == KT - 1))
                        if half is None:
                            nc.scalar.copy(out=fN[nt], in_=ps[:, 0:DIM])
                        else:
                            nc.scalar.copy(out=fN[nt][:, 0:DIM], in_=ps[:, 0:DIM])

                # ---- Q: transposed projection + per-head token stats ----
                xq_t = stream(xq)
                nc.sync.dma_start(sel, sel_d)
                nc.scalar.copy(out=sel_bf, in_=sel)
                nc.sync.dma_start(e2blk, e2_d)
                yTq = load_norm_transpose(xq_t, all_act=True)
                for jt in range(KT):
                    ps = ps_big.tile([128, N], F32, tag="big")
                    for i in range(NT):
                        for kt in range(KT):
                            nc.tensor.matmul(
                                ps[:, i * 128:(i + 1) * 128],
                                (wp[kt][:, jt * 128:(jt + 1) * 128]),
                                yTq[:, i * DIM + kt * 128:i * DIM + (kt + 1) * 128],
                                start=(kt == 0), stop=(kt == KT - 1))
                    if jt % 2 == 0:
                        nc.scalar.copy(out=fqT[jt], in_=ps)
                    else:
                        nc.vector.tensor_copy(out=fqT[jt], in_=ps)
                for jt in range(KT):
                    sq = sqp.tile([128, N], BF, tag="sq")
                    nc.gpsimd.tensor_tensor(out=sq, in0=fqT[jt], in1=fqT[jt], op=ALU.mult)
                    for hf in range(2):
                        sl = slice(hf * 512, (hf + 1) * 512)
                        ps_s = ps_stat.tile([2, 512], F32, tag="stat")
                        nc.tensor.matmul(ps_s, sel, fqT[jt][:, sl], start=True, stop=True)
                        ps_q = ps_stat.tile([2, 512], F32, tag="stat")
                        nc.tensor.matmul(ps_q, sel_bf, sq[:, sl], start=True, stop=True)
                        nc.scalar.copy(out=mqst[jt][:, sl], in_=ps_s)
                        nc.scalar.activation(nqst[jt][:, sl], ps_q, AFT.Sqrt)
                    # gate stats for q from fqT rows
                    st6b = scp.tile([128, 2, 6], F32, tag="st6b")
                    for s in range(2):
                        nc.vector.bn_stats(st6b[:, s, :], fqT[jt][:, s * 512:(s + 1) * 512])
                    mv2 = scp.tile([128, 2], F32, tag="mv2")
                    nc.vector.bn_aggr(mv2, st6b)
                    e2 = scp.tile([128, 1], F32, tag="e2")
                    nc.vector.scalar_tensor_tensor(
                        out=e2, in0=mv2[:, 0:1], scalar=mv2[:, 0:1], in1=mv2[:, 1:2],
                        op0=ALU.mult, op1=ALU.add)
                    nc.vector.tensor_copy(out=pack128[:, jt:jt + 1], in_=mv2[:, 0:1])
                    nc.vector.tensor_copy(out=pack128[:, 4 + jt:5 + jt], in_=e2)


                with nc.allow_low_precision(reason="bf16 stat rows"):
                    for jt in range(KT):
                        nc.vector.reciprocal(nqst[jt], nqst[jt])

                # ---- K: normal-orientation projection + token stats + gate sums ----
                xk_t = stream(xk)
                nc.sync.dma_start(ones_col, ones_d)
                nc.sync.dma_start(gpk, gpk_d)
                xv_t = preload(xv, "v")
                yTk = load_norm_transpose(xk_t)
                project_N(yTk, fk)
                ps_ks = ps_stat.tile([1, DIM], F32, tag="stat")
                ps_kq = ps_stat.tile([1, DIM], F32, tag="stat")
                for nt in range(NT):
                    sqk = sqp.tile([128, DIM], BF, tag="sqk")
                    nc.scalar.square(sqk, fk[nt])
                    nk2 = scp.tile([128, H], F32, tag="nk2")
                    nc.vector.tensor_reduce(out=nk2, in_=sqk.rearrange(
                        "p (h d) -> p h d", h=H), axis=mybir.AxisListType.X, op=ALU.add)
                    nk = scp.tile([128, H], F32, tag="nk")
                    nc.scalar.activation(nk, nk2, AFT.Sqrt)
                    nc.vector.reciprocal(invnk[nt], nk)
                    # gate sums for k (column sums over tokens)
                    nc.tensor.matmul(ps_ks, ones_colbf, fk[nt],
                                     start=(nt == 0), stop=(nt == NT - 1))
                    nc.tensor.matmul(ps_kq, ones_colbf, sqk,
                                     start=(nt == 0), stop=(nt == NT - 1))
                kmean_row = scp.tile([1, DIM], F32, tag="kmean_row", bufs=1)
                nc.vector.tensor_scalar(out=kmean_row, in0=ps_ks, scalar1=1.0 / 1024.0,
                                        scalar2=None, op0=ALU.mult)
                ke2_row = scp.tile([1, DIM], F32, tag="ke2_row", bufs=1)
                nc.vector.tensor_scalar(out=ke2_row, in0=ps_kq, scalar1=1.0 / 1024.0,
                                        scalar2=None, op0=ALU.mult)
                for t in range(KT):
                    nc.sync.dma_start(pack128[:, 8 + t:9 + t],
                                      kmean_row[0:1, t * 128:(t + 1) * 128])
                    nc.sync.dma_start(pack128[:, 12 + t:13 + t],
                                      ke2_row[0:1, t * 128:(t + 1) * 128])

                # ---- collective allreduce of gate stats ----
                rv = red_in[:].rearrange("d (c t two) -> d c t two", c=4, t=4, two=2)
                nc.sync.dma_start(rv[:, :, :, 0],
                                  pack128[0:64, :].rearrange("d (c t) -> d c t", c=4))
                nc.sync.dma_start(rv[:, :, :, 1],
                                  pack128[64:128, :].rearrange("d (c t) -> d c t", c=4))
                if single_core:
                    nc.gpsimd.dma_start(red_out[:], red_in[:])
                else:
                    nc.gpsimd.collective_compute(
                        "AllReduce", ALU.add,
                        replica_groups=[list(range(N_CORES))],
                        ins=[red_in[:].opt()], outs=[red_out[:].opt()])
                ro = red_out[:].rearrange("d (g e) -> d g e", g=4)
                nc.sync.dma_start(mlp_raw[0:64, :], ro[:, 0, :])
                nc.sync.dma_start(mlp_raw[64:128, :], ro[:, 2, :])
                nc.sync.dma_start(e2_raw[0:64, :], ro[:, 1, :])
                nc.sync.dma_start(e2_raw[64:128, :], ro[:, 3, :])

                # ---- gate math ----
                gsc = scp
                mlp_in = persist.tile([128, H], F32)
                nc.vector.tensor_scalar(out=mlp_in, in0=mlp_raw, scalar1=1.0 / 8.0,
                                        scalar2=None, op0=ALU.mult)
                mu_d = gsc.tile([128, 1], F32, tag="g1")
                nc.vector.tensor_reduce(out=mu_d, in_=mlp_raw, axis=mybir.AxisListType.X,
                                        op=ALU.add)
                nc.vector.tensor_scalar(out=mu_d, in0=mu_d, scalar1=1.0 / 64.0,
                                        scalar2=None, op0=ALU.mult)
                msq_d = gsc.tile([128, 1], F32, tag="g2")
                nc.vector.tensor_reduce(out=msq_d, in_=e2_raw, axis=mybir.AxisListType.X,
                                        op=ALU.add)
                nc.vector.tensor_scalar(out=msq_d, in0=msq_d, scalar1=1.0 / 64.0,
                                        scalar2=None, op0=ALU.mult)
                var_d = gsc.tile([128, 1], F32, tag="g3")
                nc.vector.scalar_tensor_tensor(out=var_d, in0=mu_d, scalar=mu_d,
                                               in1=msq_d, op0=ALU.mult, op1=ALU.subtract)
                nc.vector.tensor_scalar(out=var_d, in0=var_d,
                                        scalar1=-(M_TOT / (M_TOT - 1.0)),
                                        scalar2=None, op0=ALU.mult)
                std_d = gsc.tile([128, 1], F32, tag="g4")
                nc.scalar.activation(std_d, var_d, AFT.Sqrt, bias=eps_var_t)
                pen = gsc.tile([128, 1], F32, tag="g5")
                nc.scalar.activation(pen, std_d, AFT.Relu, bias=1.0, scale=-1.0)
                vsum = gsc.tile([128, 1], F32, tag="g6")
                nc.gpsimd.partition_all_reduce(vsum, pen, channels=128,
                                               reduce_op=bass_isa.ReduceOp.add)
                ps_h1 = ps_stat.tile([D, H], F32, tag="stat")
                nc.tensor.matmul(ps_h1, pw1, mlp_in, start=True, stop=True)
                h1 = gsc.tile([D, H], F32, tag="h1")
                nc.vector.tensor_scalar(out=h1, in0=ps_h1, scalar1=pb1, scalar2=None,
                                        op0=ALU.add)
                s1 = gsc.tile([D, H], F32, tag="s1")
                nc.gpsimd.partition_all_reduce(s1, h1, channels=D,
                                               reduce_op=bass_isa.ReduceOp.add)
                h1sq = gsc.tile([D, H], F32, tag="h1sq")
                nc.vector.tensor_tensor(out=h1sq, in0=h1, in1=h1, op=ALU.mult)
                s2 = gsc.tile([D, H], F32, tag="s2")
                nc.gpsimd.partition_all_reduce(s2, h1sq, channels=D,
                                               reduce_op=bass_isa.ReduceOp.add)
                mean_g = gsc.tile([D, H], F32, tag="mg")
                nc.vector.tensor_scalar(out=mean_g, in0=s1, scalar1=1.0 / 64.0,
                                        scalar2=None, op0=ALU.mult)
                var_g = gsc.tile([D, H], F32, tag="vg")
                nc.vector.tensor_tensor(out=var_g, in0=mean_g, in1=mean_g, op=ALU.mult)
                nc.vector.scalar_tensor_tensor(out=var_g, in0=s2, scalar=1.0 / 64.0,
                                               in1=var_g, op0=ALU.mult, op1=ALU.subtract)
                sdg = gsc.tile([D, H], F32, tag="sdg")
                nc.scalar.activation(sdg, var_g, AFT.Sqrt, bias=eps_ln_t[0:64, :])
                rst = gsc.tile([D, H], F32, tag="rst")
                nc.vector.reciprocal(rst, sdg)
                h1n = gsc.tile([D, H], F32, tag="h1n")
                nc.vector.tensor_tensor(out=h1n, in0=h1, in1=mean_g, op=ALU.subtract)
                nc.vector.tensor_tensor(out=h1n, in0=h1n, in1=rst, op=ALU.mult)
                nc.vector.tensor_scalar(out=h1n, in0=h1n, scalar1=plng, scalar2=plnb,
                                        op0=ALU.mult, op1=ALU.add)
                nc.scalar.activation(h1n, h1n, AFT.Relu)
                ps_h2 = ps_stat.tile([1, H], F32, tag="stat")
                nc.tensor.matmul(ps_h2, pw2, h1n, start=True, stop=True)
                sig = gsc.tile([1, H], F32, tag="sig")
                nc.scalar.activation(sig, ps_h2, AFT.Sigmoid, bias=pb2)
                onep = gsc.tile([1, 1], F32, tag="onep")
                nc.vector.tensor_scalar(out=onep, in0=vsum[0:1, :], scalar1=1.0 / 64.0,
                                        scalar2=1.0, op0=ALU.mult, op1=ALU.add)
                inv1p = gsc.tile([1, 1], F32, tag="inv1p")
                nc.vector.reciprocal(inv1p, onep)
                wrow = gsc.tile([1, H], F32, tag="wrow")
                nc.vector.tensor_scalar(out=wrow, in0=sig, scalar1=inv1p, scalar2=None,
                                        op0=ALU.mult)
                w64row = gsc.tile([1, H], F32, tag="w64row")
                nc.vector.tensor_scalar(out=w64row, in0=wrow, scalar1=1.0 / 64.0,
                                        scalar2=None, op0=ALU.mult)
                omrow = gsc.tile([1, H], F32, tag="omrow")
                nc.vector.tensor_scalar(out=omrow, in0=wrow, scalar1=-1.0, scalar2=1.0,
                                        op0=ALU.mult, op1=ALU.add)
                nc.gpsimd.partition_broadcast(w64_bc, w64row, channels=128)
                nc.gpsimd.partition_broadcast(om_bc, omrow, channels=128)
                for jt in range(KT):
                    nc.vector.tensor_copy(out=omx[0:64, jt:jt + 1],
                                          in_=om_bc[0:64, 2 * jt:2 * jt + 1])
                    nc.scalar.copy(out=omx[64:128, jt:jt + 1],
                                   in_=om_bc[64:128, 2 * jt + 1:2 * jt + 2])
                    nc.vector.tensor_copy(out=w64x[0:64, jt:jt + 1],
                                          in_=w64_bc[0:64, 2 * jt:2 * jt + 1])
                    nc.scalar.copy(out=w64x[64:128, jt:jt + 1],
                                   in_=w64_bc[64:128, 2 * jt + 1:2 * jt + 2])

                # ---- V ----
                yTv = load_norm_transpose(xv_t)
                project_N(yTv, fvall, half=True)
                for nt in range(NT):
                    for h in range(H):
                        eng = nc.vector if h % 2 == 0 else nc.gpsimd
                        eng.tensor_scalar(
                            out=fvall[nt][:, DIM + h * 64:DIM + (h + 1) * 64],
                            in0=fvall[nt][:, h * 64:(h + 1) * 64],
                            scalar1=invnk[nt][:, h:h + 1],
                            scalar2=None, op0=ALU.mult)

            if debug:
                dbg_f32 = persist.tile([128, 2 * DIM], F32)
                nc.vector.tensor_copy(out=dbg_f32[:, 0:N], in_=fqT[0])
                nc.sync.dma_start(dbg_fq, dbg_f32[:, 0:N])
                nc.vector.tensor_copy(out=dbg_f32[:, 0:DIM], in_=fk[0])
                nc.sync.dma_start(dbg_fk, dbg_f32[:, 0:DIM])
                nc.vector.tensor_copy(out=dbg_f32, in_=fvall[0])
                nc.sync.dma_start(dbg_fv, dbg_f32)
                dbg_s = persist.tile([2, N], F32)
                nc.vector.tensor_copy(out=dbg_s, in_=nqst[0])
                nc.sync.dma_start(dbg_nq, dbg_s)
                dbg_s2 = persist.tile([2, N], F32)
                nc.vector.tensor_copy(out=dbg_s2, in_=mqst[0])
                nc.sync.dma_start(dbg_mq, dbg_s2)

            # ================= PHASE B =================
            with tc.tile_pool(name="prp", bufs=2) as prp, \
                 tc.tile_pool(name="fqp", bufs=3) as fqp, \
                 tc.tile_pool(name="ps_pr", bufs=2, space="PSUM") as ps_pr, \
                 tc.tile_pool(name="ps_c", bufs=1, space="PSUM") as ps_c, \
                 tc.tile_pool(name="ps_bc", bufs=1, space="PSUM") as ps_bc, \
                 tc.tile_pool(name="ps_o2", bufs=2, space="PSUM") as ps_o2:

                for jt in range(KT):  # head pair
                    pr_ps = ps_pr.tile([128, 256], F32, tag="pr")
                    for c in range(NT):
                        rhs = fvall[c].rearrange("p (s j) -> p s j", s=2)[
                            :, :, jt * 128:(jt + 1) * 128]
                        nc.tensor.matmul(pr_ps, fk[c][:, jt * 128:(jt + 1) * 128],
                                         rhs, start=(c == 0), stop=(c == NT - 1))
                    # pr_ps cols 0:128 = fk_pair^T fv_pair (R), 128:256 = ^T fvs_pair (P);
                    # diagonal 64x64 quadrants valid, crosses garbage.
                    # Block-diagonal P (scaled by om) and R (scaled by w/64).
                    pblk = prp.tile([128, 128], F32R, tag="pblk")
                    rblk = prp.tile([128, 128], F32R, tag="rblk")
                    nc.vector.tensor_scalar(out=pblk[0:64, 0:64], in0=pr_ps[0:64, 128:192],
                                            scalar1=omx[0:64, jt:jt + 1],
                                            scalar2=None, op0=ALU.mult)
                    nc.vector.tensor_scalar(out=pblk[64:128, 64:128],
                                            in0=pr_ps[64:128, 192:256],
                                            scalar1=omx[64:128, jt:jt + 1],
                                            scalar2=None, op0=ALU.mult)
                    nc.scalar.copy(out=pblk[0:64, 64:128], in_=zeros64)
                    nc.scalar.copy(out=pblk[64:128, 0:64], in_=zeros64)
                    nc.vector.tensor_scalar(out=rblk[0:64, 0:64], in0=pr_ps[0:64, 0:64],
                                            scalar1=w64x[0:64, jt:jt + 1],
                                            scalar2=None, op0=ALU.mult)
                    nc.vector.tensor_scalar(out=rblk[64:128, 64:128],
                                            in0=pr_ps[64:128, 64:128],
                                            scalar1=w64x[64:128, jt:jt + 1],
                                            scalar2=None, op0=ALU.mult)
                    nc.scalar.copy(out=rblk[0:64, 64:128], in_=zeros64)
                    nc.scalar.copy(out=rblk[64:128, 0:64], in_=zeros64)
                    # c-pair rows = -colsum(w64*R)/64 (gate scale inherited)
                    cps = ps_c.tile([2, 128], F32, tag="c")
                    nc.tensor.matmul(cps, sel, rblk, start=True, stop=True)
                    cpair = prp.tile([2, 128], F32R, tag="cpair")
                    nc.vector.tensor_scalar(out=cpair, in0=cps, scalar1=-1.0 / 64.0,
                                            scalar2=None, op0=ALU.mult)
                    # per-head 1/nq rows -> [128, 512] chunks via block-indicator matmul
                    fqn = fqp.tile([128, N], F32R, tag="fqn")
                    for hf in range(2):
                        sl = slice(hf * 512, (hf + 1) * 512)
                        bc = ps_bc.tile([128, 512], F32, tag="bc")
                        nc.tensor.matmul(bc, e2blk, nqst[jt][:, sl],
                                         start=True, stop=True)
                        nc.vector.tensor_tensor(out=fqn[:, sl], in0=fqT[jt][:, sl],
                                                in1=bc, op=ALU.mult)
                    # single accumulation: om*cos + w*cov, then one bf16 copy out
                    o2 = ps_o2.tile([128, N], F32, tag="o2")
                    for hf in range(2):
                        sl = slice(hf * 512, (hf + 1) * 512)
                        nc.tensor.matmul(o2[:, sl], pblk, fqn[:, sl],
                                         start=True, stop=False)
                        nc.tensor.matmul(o2[:, sl], rblk, fqT[jt][:, sl],
                                         start=False, stop=False, skip_group_check=True)
                        nc.tensor.matmul(o2[:, sl], cpair, mqst[jt][:, sl],
                                         start=False, stop=True, skip_group_check=True)
                    nc.scalar.copy(out=GTb[jt], in_=o2)

            if debug:
                dbg_g = persist.tile([128, N], F32)
                nc.vector.tensor_copy(out=dbg_g, in_=GTb[0])
                nc.sync.dma_start(dbg_gtb, dbg_g)

            # ================= PHASE C =================
            with tc.tile_pool(name="op", bufs=6) as op_pool, \
                 tc.tile_pool(name="wop", bufs=1) as wop, \
                 tc.tile_pool(name="ps_out", bufs=4, space="PSUM") as ps_out:
                nc.sync.dma_start(bout_row, bout_d)
                nc.scalar.copy(out=bout_bf, in_=bout_row)
                wo_all = wop.tile([128, KT * DIM], BF)
                nc.sync.dma_start(wo_all, wout_d)
                wo = [wo_all[:, t * DIM:(t + 1) * DIM] for t in range(KT)]
                for nt in range(NT):
                    ps = ps_out.tile([128, DIM], F32, tag="o")
                    for kt in range(KT):
                        nc.tensor.matmul(ps, GTb[kt][:, nt * 128:(nt + 1) * 128],
                                         wo[kt], start=(kt == 0), stop=False)
                    nc.tensor.matmul(ps, ones_row_bf, bout_bf, start=False, stop=True,
                                     skip_group_check=True)
                    o_sb = op_pool.tile([128, DIM], F32, tag="osb")
                    nc.vector.tensor_copy(out=o_sb, in_=ps)
                    nc.sync.dma_start(out_d[nt * 128:(nt + 1) * 128, :], o_sb)

    nc.compile()
    return nc


_NC_CACHE = None


def _get_nc():
    global _NC_CACHE
    if _NC_CACHE is None:
        _NC_CACHE = build_bass()
    return _NC_CACHE


def kernel(q, k, v, ln_g, ln_b, w_in, p_w1, p_b1, p_ln_g, p_ln_b, p_w2, p_b2,
           w_out, b_out, **extra):
    q = np.asarray(q, np.float32); k = np.asarray(k, np.float32); v = np.asarray(v, np.float32)
    ln_g = np.asarray(ln_g, np.float32); ln_b = np.asarray(ln_b, np.float32)
    w_in = np.asarray(w_in, np.float32)
    wp = (ln_g[:, None] * w_in).astype(np.float32)
    cb = (ln_b @ w_in).astype(np.float32)
    assert np.abs(cb).max() == 0.0, "kernel fast path assumes ln_b == 0"
    sel = np.zeros((128, 2), np.float32)
    sel[0:64, 0] = 1.0
    sel[64:128, 1] = 1.0
    gate_pack = np.zeros((128, 72), np.float32)
    gate_pack[:, 0:64] = np.asarray(p_w1, np.float32)
    gate_pack[0:64, 64] = np.asarray(p_b1, np.float32).reshape(-1)
    gate_pack[0:64, 65] = np.asarray(p_ln_g, np.float32).reshape(-1)
    gate_pack[0:64, 66] = np.asarray(p_ln_b, np.float32).reshape(-1)
    gate_pack[0:64, 67] = np.asarray(p_w2, np.float32).reshape(-1)
    gate_pack[0, 68] = float(np.asarray(p_b2).reshape(-1)[0])
    shared = {
        "wp": np.concatenate([wp[t * 128:(t + 1) * 128, :] for t in range(4)],
                             axis=1).astype(ml_dtypes.bfloat16),
        "cb_col": cb.reshape(DIM, 1).copy(),
        "cb_row": cb.reshape(1, DIM).copy(),
        "wout": np.concatenate(
            [np.asarray(w_out, np.float32)[t * 128:(t + 1) * 128, :] for t in range(4)],
            axis=1).astype(ml_dtypes.bfloat16),
        "gate_pack": gate_pack,
        "bout_row": np.asarray(b_out, np.float32).reshape(1, DIM).copy(),

        "sel": sel,
        "ones_col": np.ones((128, 1), np.float32),
        "e2blk": np.kron(np.eye(2), np.ones((1, 64))).astype(ml_dtypes.bfloat16),
    }
    in_maps = []
    for g in range(N_CORES):
        m = dict(shared)
        m["xq"] = np.ascontiguousarray(q[g])
        m["xk"] = np.ascontiguousarray(k[g])
        m["xv"] = np.ascontiguousarray(v[g])
        in_maps.append(m)
    nc = _get_nc()
    res = run_bass_kernel_spmd(nc, in_maps, core_ids=list(range(N_CORES)))
    out = np.stack([res.results[g]["out"] for g in range(N_CORES)], axis=0)
    return out

